# revision 1
# baseline (speedup 1.0000x reference)
"""Trainium2 Bass kernel for nn_MedPoseAttention (multi-head cross-attention).

Full inputs in, full outputs out. Sharding: 8 cores = 4 batches x 2 query-row
halves. Each core computes one batch's K/V projections over the full context
(replicated within the pair -- cheaper than any collective on this hardware)
and attention + output projection for its 512 query rows, all 16 heads.
No cross-core communication.

Per-core dataflow (all matmuls fp32r / tf32-rate):
  qT/kT  = W.T @ X^T per head-pair          [128=2x64hd, L]   (weights stationary)
  scoresT = kT.T @ qT per head              [kv, q] via 2-head row-tiling (K=64)
  expT   = exp(0.125 * scoresT)             ScalarE, PSUM->SBUF, fp32r
  pv     = [v | 1].T @ expT                 M=65: rows 0-63 = head_out^T,
                                            row 64 = softmax denominator
  multiT = pv[0:64] * bcast(1/denom)        DVE mult, gpsimd partition_broadcast
  out    = multiT.T @ Wo + bo               multiT stationary, bias via K=1 matmul
"""

import sys

if "/opt/trn_rl_repo" not in sys.path:
    sys.path.insert(0, "/opt/trn_rl_repo")

import numpy as np

import concourse.bass as bass  # noqa: F401
import concourse.mybir as mybir
from concourse import bacc, tile
from concourse.bass_utils import run_bass_kernel_spmd

F32 = mybir.dt.float32
F32R = mybir.dt.float32r
MULT = mybir.AluOpType.mult
EXP = mybir.ActivationFunctionType.Exp

B, L, D, H, HD = 4, 1024, 1024, 16, 64
NCORES = 8
LQ_C = 512  # query rows per core
NP = H // 2  # head pairs
SCALE = 0.125  # 1/sqrt(HD)

_PROGRAM = None


def build_program(reps=1):
    nc = bacc.Bacc("TRN2", target_bir_lowering=False, debug=False, num_devices=NCORES)

    xqt = nc.dram_tensor("xqt", [D, LQ_C], F32, kind="ExternalInput").ap()
    xct = nc.dram_tensor("xct", [D, L], F32, kind="ExternalInput").ap()
    wq = nc.dram_tensor("wq", [D, H * HD], F32, kind="ExternalInput").ap()
    wk = nc.dram_tensor("wk", [D, H * HD], F32, kind="ExternalInput").ap()
    wv = nc.dram_tensor("wv", [D, H * 65], F32, kind="ExternalInput").ap()
    wo = nc.dram_tensor("wo", [H * HD, D], F32, kind="ExternalInput").ap()
    bq = nc.dram_tensor("bq", [128, NP], F32, kind="ExternalInput").ap()
    bk = nc.dram_tensor("bk", [128, NP], F32, kind="ExternalInput").ap()
    bvrow = nc.dram_tensor("bvrow", [1, H * 65], F32, kind="ExternalInput").ap()
    borow = nc.dram_tensor("borow", [1, D], F32, kind="ExternalInput").ap()
    onesd = nc.dram_tensor("onesd", [1, 128], F32, kind="ExternalInput").ap()

    out = nc.dram_tensor("out", [LQ_C, D], F32, kind="ExternalOutput").ap()
    resid = nc.dram_tensor("resid", [H * HD, LQ_C], F32, kind="ExternalOutput").ap()

    # DRAM views with the d/hc blocking on the partition axis: [128, 8, m]
    wq_v = wq.rearrange("(db p) m -> p db m", p=128)
    wk_v = wk.rearrange("(db p) m -> p db m", p=128)
    wv_v = wv.rearrange("(db p) m -> p db m", p=128)
    wo_v = wo.rearrange("(hb p) m -> p hb m", p=128)

    with tile.TileContext(nc) as tc:
        with (
            tc.tile_pool(name="persist", bufs=1) as persist,
            tc.tile_pool(name="wqk", bufs=3) as wqk_pool,
            tc.tile_pool(name="wbig", bufs=2) as wbig_pool,
            tc.tile_pool(name="qtp", bufs=2) as qt_pool,
            tc.tile_pool(name="ktp", bufs=2) as kt_pool,
            tc.tile_pool(name="expp", bufs=7) as exp_pool,
            tc.tile_pool(name="smallp", bufs=2) as small,
            tc.tile_pool(name="psA", bufs=2, space="PSUM") as psA,
            tc.tile_pool(name="psS", bufs=2, space="PSUM") as psS,
            tc.tile_pool(name="psV", bufs=2, space="PSUM") as psV,
        ):
            def emit_preamble():
                ones_r = persist.tile([1, 128], F32R, tag="ones", name="ones_r")
                bq_sb = persist.tile([128, NP], F32, tag="bq", name="bq_sb")
                bk_sb = persist.tile([128, NP], F32, tag="bk", name="bk_sb")
                bvr = persist.tile([1, H * 65], F32R, tag="bvr", name="bvr")
                bor = persist.tile([1, D], F32R, tag="bor", name="bor")
                xq_all = persist.tile([128, 8, LQ_C], F32R, tag="xq", name="xq_all")
                xqt_v = xqt.rearrange("(db p) m -> p db m", p=128).bitcast(F32R)
                nc.scalar.dma_start(xq_all[:, 0:4, :], xqt_v[:, 0:4, :])
                xq_t = [xq_all[:, d, :] for d in range(8)]
                multi = [
                    persist.tile([128, LQ_C], F32R, tag=f"mt{p}", name=f"mt{p}")
                    for p in range(NP)
                ]
                xc_all = persist.tile([128, 8, L], F32R, tag="xc", name="xc_all")
                v_all = persist.tile([128, 8 * H * 65], F32R, tag="vb", name="v_all")
                return ones_r, bq_sb, bk_sb, bvr, bor, xq_t, multi, xc_all, v_all, xq_all, xqt_v

            def emit_iteration():
                (
                    ones_r, bq_sb, bk_sb, bvr, bor, xq_t, multi, xc_all, v_all,
                    xq_all, xqt_v,
                ) = emit_preamble()
                qT = [None] * NP
                kT = [None] * NP
                expT = [[None] * 8 for _ in range(NP)]
                xc_t = []
                v_buf = []

                def emit_small_consts():
                    nc.sync.dma_start(bq_sb[:], bq[:])
                    nc.sync.dma_start(bk_sb[:], bk[:])

                def emit_late_consts():
                    nc.sync.dma_start(ones_r[:], onesd[:].bitcast(F32R))
                    nc.sync.dma_start(bvr[:], bvrow[:].bitcast(F32R))
                    nc.sync.dma_start(bor[:], borow[:].bitcast(F32R))

                def emit_q(p):
                    wqt = wqk_pool.tile([128, 8, 128], F32R, tag="wqk", name=f"wq{p}")
                    nc.sync.dma_start(wqt[:], wq_v[:, :, p * 128 : (p + 1) * 128].bitcast(F32R))
                    nc.scalar.dma_start(
                        xq_all[:, 4:8, :], xqt_v[:, 4:8, :]
                    )
                    qps = psA.tile([128, LQ_C], F32, tag="proj", name=f"qps{p}")
                    for d in range(8):
                        nc.tensor.matmul(
                            qps[:],
                            lhsT=wqt[:, d, :],
                            rhs=xq_t[d][:],
                            start=(d == 0),
                            stop=(d == 7),
                        )
                    finish_q(p, qps)

                def finish_q(p, qps):
                    qT[p] = qt_pool.tile([128, LQ_C], F32R, tag="qt", name=f"qt{p}")
                    nc.vector.tensor_scalar_add(qT[p][:], qps[:], bq_sb[:, p : p + 1])
                    nc.sync.dma_start(
                        resid[p * 128 : (p + 1) * 128, :], qT[p][:].bitcast(F32)
                    )

                def emit_xct():
                    # context^T split across the gpsimd/SWDGE and scalar/HWDGE
                    # rings, in two merged DMAs.
                    xct_v = xct.rearrange("(db p) m -> p db m", p=128).bitcast(F32R)
                    nc.gpsimd.dma_start(xc_all[:, 0:2, :], xct_v[:, 0:2, :])
                    nc.gpsimd.dma_start(xc_all[:, 2:4, :], xct_v[:, 2:4, :])
                    nc.scalar.dma_start(xc_all[:, 4:6, :], xct_v[:, 4:6, :])
                    nc.scalar.dma_start(xc_all[:, 6:8, :], xct_v[:, 6:8, :])
                    xc_t.extend(xc_all[:, d, :] for d in range(8))
                    v_buf.extend(
                        v_all[:, kvb * H * 65 : (kvb + 1) * H * 65] for kvb in range(8)
                    )

                def emit_k(p):
                    wkt = wqk_pool.tile([128, 8, 128], F32R, tag="wqk", name=f"wk{p}")
                    nc.sync.dma_start(wkt[:], wk_v[:, :, p * 128 : (p + 1) * 128].bitcast(F32R))
                    kT[p] = kt_pool.tile([128, L], F32R, tag="kt", name=f"kt{p}")
                    for ch in range(2):
                        kps = psA.tile([128, 512], F32, tag="proj", name=f"kps{p}{ch}")
                        for d in range(8):
                            nc.tensor.matmul(
                                kps[:],
                                lhsT=wkt[:, d, :],
                                rhs=xc_t[d][:, ch * 512 : (ch + 1) * 512],
                                start=(d == 0),
                                stop=(d == 7),
                            )
                        nc.vector.tensor_scalar_add(
                            kT[p][:, ch * 512 : (ch + 1) * 512], kps[:], bk_sb[:, p : p + 1]
                        )

                def emit_norm(p, h, pvp):
                    dstage = small.tile([65, 512], F32, tag="dstage", name=f"ds{p}{h}", bufs=1)
                    nc.vector.tensor_copy(dstage[64:65, :], pvp[64:65, :])
                    drow = small.tile([1, 512], F32, tag="drow", name=f"dr{p}{h}", bufs=1)
                    nc.sync.dma_start(drow[:], dstage[64:65, :])
                    rrow = small.tile([1, 512], F32R, tag="rrow", name=f"rr{p}{h}", bufs=1)
                    nc.vector.reciprocal(rrow[:], drow[:])
                    R = small.tile([64, 512], F32R, tag="R", name=f"R{p}{h}")
                    nc.gpsimd.partition_broadcast(R[:], rrow[:])
                    if h == 0:
                        nc.vector.tensor_tensor(
                            multi[p][0:64, :], pvp[0:64, :], R[:], op=MULT
                        )
                    else:
                        tmp = small.tile([64, 512], F32R, tag="tmp", name=f"tp{p}{h}")
                        nc.vector.tensor_tensor(tmp[:], pvp[0:64, :], R[:], op=MULT)
                        nc.sync.dma_start(multi[p][64:128, :], tmp[:])

                def emit_sblock(p, pv_pair=None, next_pair=None, pv2_pair=None):
                    pvt = []
                    pvt2 = []
                    if pv2_pair is not None:
                        # last pair's PV rides in psA (idle: no next-pair
                        # projections) at a 2-block lag behind exp
                        pvt2 = [
                            psA.tile([128, LQ_C], F32, tag="proj", name=f"pv{pv2_pair}{h}")
                            for h in range(2)
                        ]
                    if pv_pair is not None:
                        pvt = [
                            psV.tile([128, LQ_C], F32, tag="pv", name=f"pv{pv_pair}{h}")
                            for h in range(2)
                        ]
                    if next_pair is not None:
                        nx = next_pair
                        wqt = wqk_pool.tile([128, 8, 128], F32R, tag="wqk", name=f"wq{nx}")
                        nc.sync.dma_start(
                            wqt[:], wq_v[:, :, nx * 128 : (nx + 1) * 128].bitcast(F32R)
                        )
                        wkt = wqk_pool.tile([128, 8, 128], F32R, tag="wqk", name=f"wk{nx}")
                        nc.sync.dma_start(
                            wkt[:], wk_v[:, :, nx * 128 : (nx + 1) * 128].bitcast(F32R)
                        )
                        qps = psA.tile([128, LQ_C], F32, tag="proj", name=f"qps{nx}")
                        kT[nx] = kt_pool.tile([128, L], F32R, tag="kt", name=f"kt{nx}")
                        kps = [
                            psA.tile([128, 512], F32, tag="proj", name=f"kps{nx}{c}")
                            for c in range(2)
                        ]
                    for kvb in range(8):
                        sps = psS.tile([128, 1024], F32, tag="sps", name=f"sps{p}{kvb}")
                        nc.tensor.matmul(
                            sps[:, 0:512],
                            lhsT=kT[p][0:64, kvb * 128 : (kvb + 1) * 128],
                            rhs=qT[p][0:64, :],
                            start=True,
                            stop=True,
                            tile_position=(0, 0),
                        )
                        nc.tensor.matmul(
                            sps[:, 512:1024],
                            lhsT=kT[p][64:128, kvb * 128 : (kvb + 1) * 128],
                            rhs=qT[p][64:128, :],
                            start=True,
                            stop=True,
                            tile_position=(64, 0),
                        )
                        et = exp_pool.tile([128, 1024], F32R, tag="expt", name=f"et{p}{kvb}")
                        nc.scalar.activation(et[:], sps[:], EXP, scale=SCALE)
                        expT[p][kvb] = et
                        if pv_pair is not None:
                            for h in range(2):
                                hg = 2 * pv_pair + h
                                nc.tensor.matmul(
                                    pvt[h][0:65, :],
                                    lhsT=v_buf[kvb][:, hg * 65 : hg * 65 + 65],
                                    rhs=expT[pv_pair][kvb][:, h * 512 : (h + 1) * 512],
                                    start=(kvb == 0),
                                    stop=(kvb == 7),
                                )
                        if pv2_pair is not None and kvb >= 2:
                            for h in range(2):
                                hg2 = 2 * pv2_pair + h
                                nc.tensor.matmul(
                                    pvt2[h][0:65, :],
                                    lhsT=v_buf[kvb - 2][:, hg2 * 65 : hg2 * 65 + 65],
                                    rhs=expT[pv2_pair][kvb - 2][:, h * 512 : (h + 1) * 512],
                                    start=(kvb == 2),
                                    stop=False,
                                )
                        if next_pair is not None:
                            # one q matmul + two k matmuls of the next pair per step
                            nc.tensor.matmul(
                                qps[:],
                                lhsT=wqt[:, kvb, :],
                                rhs=xq_t[kvb][:],
                                start=(kvb == 0),
                                stop=(kvb == 7),
                            )
                            for j in range(2):
                                ch, d = divmod(2 * kvb + j, 8)
                                ch, d = (0, 2 * kvb + j) if kvb < 4 else (1, 2 * kvb + j - 8)
                                nc.tensor.matmul(
                                    kps[ch][:],
                                    lhsT=wkt[:, d, :],
                                    rhs=xc_t[d][:, ch * 512 : (ch + 1) * 512],
                                    start=(d == 0),
                                    stop=(d == 7),
                                )
                                if d == 7:
                                    nc.vector.tensor_scalar_add(
                                        kT[nx][:, ch * 512 : (ch + 1) * 512],
                                        kps[ch][:],
                                        bk_sb[:, nx : nx + 1],
                                    )
                    if next_pair is not None:
                        finish_q(nx, qps)
                    if pv_pair is not None:
                        for h in range(2):
                            emit_norm(pv_pair, h, pvt[h])
                    if pv2_pair is not None:
                        for kk in (6, 7):
                            for h in range(2):
                                hg2 = 2 * pv2_pair + h
                                nc.tensor.matmul(
                                    pvt2[h][0:65, :],
                                    lhsT=v_buf[kk][:, hg2 * 65 : hg2 * 65 + 65],
                                    rhs=expT[pv2_pair][kk][:, h * 512 : (h + 1) * 512],
                                    start=False,
                                    stop=(kk == 7),
                                )
                        for h in (1, 0):
                            emit_norm(pv2_pair, h, pvt2[h])

                def emit_vproj():
                    # Wv is host-augmented to [D, 16*65]: per head 64 cols + a zero
                    # column whose bias is 1.0 -> projection emits [v | 1] slots
                    # directly, softmax denominators ride the PV matmul for free.
                    for ch in range(4):
                        wvt = wbig_pool.tile([128, 8, 260], F32R, tag="wbig", name=f"wv{ch}")
                        nc.sync.dma_start(
                            wvt[:], wv_v[:, :, ch * 260 : (ch + 1) * 260].bitcast(F32R)
                        )
                        for kvb in range(8):
                            vps = psA.tile([128, 260], F32, tag="proj", name=f"vps{ch}{kvb}")
                            for d in range(8):
                                nc.tensor.matmul(
                                    vps[:],
                                    lhsT=xc_t[d][:, kvb * 128 : (kvb + 1) * 128],
                                    rhs=wvt[:, d, :],
                                    start=(d == 0),
                                    stop=False,
                                )
                            nc.tensor.matmul(
                                vps[:],
                                lhsT=ones_r[0:1, :],
                                rhs=bvr[0:1, ch * 260 : (ch + 1) * 260],
                                start=False,
                                stop=True,
                            )
                            nc.vector.tensor_copy(
                                v_all[:, kvb * 1040 + ch * 260 : kvb * 1040 + (ch + 1) * 260],
                                vps[:],
                            )

                wo_t = [None, None]

                def emit_wo_loads():
                    for ch in range(2):
                        wo_t[ch] = wbig_pool.tile(
                            [128, 8, 512], F32R, tag="wbig", name=f"wo{ch}"
                        )
                        nc.gpsimd.dma_start(
                            wo_t[ch][:], wo_v[:, :, ch * 512 : (ch + 1) * 512].bitcast(F32R)
                        )

                def emit_pv(p):
                    for h in (1, 0):
                        hg = 2 * p + h
                        pvp = psV.tile([128, LQ_C], F32, tag="pv", name=f"pv{p}{h}")
                        for kvb in range(8):
                            nc.tensor.matmul(
                                pvp[0:65, :],
                                lhsT=v_buf[kvb][:, hg * 65 : hg * 65 + 65],
                                rhs=expT[p][kvb][:, h * 512 : (h + 1) * 512],
                                start=(kvb == 0),
                                stop=(kvb == 7),
                            )
                        emit_norm(p, h, pvp)

                def emit_oproj():
                    pools = [(psA, "proj"), (psS, "sps"), (psV, "pv")]
                    for r, (ch, lb) in enumerate(
                        [(c, l) for c in range(2) for l in range(4)]
                    ):
                        pool, tag = pools[r % 3]
                        ops = pool.tile([128, 512], F32, tag=tag, name=f"ops{ch}{lb}")
                        for hcb in range(8):
                            nc.tensor.matmul(
                                ops[:],
                                lhsT=multi[hcb][:, lb * 128 : (lb + 1) * 128],
                                rhs=wo_t[ch][:, hcb, :],
                                start=(hcb == 0),
                                stop=False,
                            )
                        nc.tensor.matmul(
                            ops[:],
                            lhsT=ones_r[0:1, :],
                            rhs=bor[0:1, ch * 512 : (ch + 1) * 512],
                            start=False,
                            stop=True,
                        )
                        osb = small.tile([128, 512], F32, tag="outsb", name=f"ob{ch}{lb}")
                        nc.vector.tensor_copy(osb[:], ops[:])
                        nc.sync.dma_start(
                            out[lb * 128 : (lb + 1) * 128, ch * 512 : (ch + 1) * 512],
                            osb[:],
                        )

                emit_small_consts()
                emit_q(0)
                emit_xct()
                emit_k(0)
                emit_late_consts()
                emit_vproj()
                emit_sblock(0, pv_pair=None, next_pair=1)
                for p in range(1, NP):
                    emit_sblock(
                        p,
                        pv_pair=p - 1,
                        next_pair=(p + 1 if p + 1 < NP else None),
                        pv2_pair=(NP - 1 if p == NP - 1 else None),
                    )
                    if p == 5:
                        emit_wo_loads()
                emit_oproj()

            with nc.allow_low_precision(reason="fp32r kernel"):
                for _rep in range(reps):
                    emit_iteration()

    nc.compile()
    return nc


def _marshal(inputs):
    q = np.ascontiguousarray(np.asarray(inputs["queries"], dtype=np.float32))
    c = np.ascontiguousarray(np.asarray(inputs["context"], dtype=np.float32))
    Wq = np.asarray(inputs["Wq"], dtype=np.float32)
    Wk = np.asarray(inputs["Wk"], dtype=np.float32)
    Wv = np.asarray(inputs["Wv"], dtype=np.float32)
    Wo = np.ascontiguousarray(np.asarray(inputs["Wo"], dtype=np.float32))
    bq = np.asarray(inputs["bq"], dtype=np.float32)
    bk = np.asarray(inputs["bk"], dtype=np.float32)
    bv = np.asarray(inputs["bv"], dtype=np.float32)
    bo = np.asarray(inputs["bo"], dtype=np.float32)

    wq_flat = np.ascontiguousarray(Wq.transpose(1, 0, 2).reshape(D, H * HD))
    wk_flat = np.ascontiguousarray(Wk.transpose(1, 0, 2).reshape(D, H * HD))
    wv_aug = np.zeros((D, H, 65), np.float32)
    wv_aug[:, :, :64] = Wv.transpose(1, 0, 2).reshape(D, H, HD)
    wv_aug = np.ascontiguousarray(wv_aug.reshape(D, H * 65))
    bv_aug = np.full((H, 65), 1.0, np.float32)
    bv_aug[:, :64] = bv.reshape(H, HD)
    bv_aug = np.ascontiguousarray(bv_aug.reshape(1, H * 65))

    bq_cols = np.ascontiguousarray(bq.reshape(NP, 128).T)
    bk_cols = np.ascontiguousarray(bk.reshape(NP, 128).T)
    shared = {
        "wq": wq_flat,
        "wk": wk_flat,
        "wv": wv_aug,
        "wo": Wo,
        "bq": bq_cols,
        "bk": bk_cols,
        "bvrow": bv_aug,
        "borow": np.ascontiguousarray(bo.reshape(1, D)),
        "onesd": np.ones((1, 128), np.float32),
    }
    in_maps = []
    for core in range(NCORES):
        b, half = core // 2, core % 2
        m = dict(shared)
        m["xqt"] = np.ascontiguousarray(q[b].T[:, half * LQ_C : (half + 1) * LQ_C])
        m["xct"] = np.ascontiguousarray(c[b].T)
        in_maps.append(m)
    return in_maps


def kernel(**inputs):
    global _PROGRAM
    if _PROGRAM is None:
        _PROGRAM = build_program()
    in_maps = _marshal(inputs)
    res = run_bass_kernel_spmd(_PROGRAM, in_maps, list(range(NCORES)))
    out = np.empty((B, L, D), np.float32)
    residual = np.empty((B, L, H * HD), np.float32)
    for core in range(NCORES):
        b, half = core // 2, core % 2
        sl = slice(half * LQ_C, (half + 1) * LQ_C)
        out[b, sl, :] = res.results[core]["out"]
        residual[b, sl, :] = res.results[core]["resid"].T
    return out, residual



# revision 2
# speedup vs baseline: 1.0315x; 1.0315x over previous
"""Trainium2 Bass kernel for nn_MedPoseAttention — v1 (delta-fp8 projections).

Sharding: 8 cores = 4 batches x 2 query halves; each core does full context
K/V (replicated within the pair), all 16 heads, 512 query rows.

Per-core dataflow:
  Q/K/V projections: two-level fp8e4m3 DoubleRow (x8*w8 + dx8*w8 + x8*dw8),
    host-prepared operands, weights pre-scaled x32. 3 passes x 4 DR matmuls
    (K=256 each) = 0.75x the bf16 matmul cost at ~bf16 accuracy.
  resid = q_psum/32 + bq  (DVE tensor_scalar, bf16 out, DMA'd directly)
  scores = qT_bf . kT_bf per head (bf16, PE quadrant packing, K=64)
  exp: ACT activation Exp -> bf16 tiles [128kv, 2heads*512q]
  PV flipped: out[q, 65] per (pair-head, q-slice); col 64 = denominator
    (v_aug 65th col = 1.0 in the f32r bias ones-row; v_bf = psum/32).
  norm: R = 1/den (per-partition!); multi = pv * R (tensor_scalar, bf16)
    = 32*(head_out + bv) -> mflip [q, hc]
  transpose: PE identity-transpose mflip -> multiT [hc, q] (bf16)
  O-proj: out[Dslice, q] = multiT.T @ Wo_bf16; out = psum/32 + bo (per-part).
Weight slices for Q/K are host-packed per head-pair in SBUF layout so each
pair's 128KB slice arrives as one clean DMA just before it is needed.
"""

import sys

if "/opt/trn_rl_repo" not in sys.path:
    sys.path.insert(0, "/opt/trn_rl_repo")

import numpy as np
import ml_dtypes

import concourse.bass as bass  # noqa: F401
import concourse.mybir as mybir
from concourse import bacc, tile
from concourse.bass_utils import run_bass_kernel_spmd

F32 = mybir.dt.float32
F32R = mybir.dt.float32r
BF16 = mybir.dt.bfloat16
F8 = mybir.dt.float8e4
DR = mybir.MatmulPerfMode.DoubleRow
MULT = mybir.AluOpType.mult
ADD = mybir.AluOpType.add
EXP = mybir.ActivationFunctionType.Exp

E4NP = ml_dtypes.float8_e4m3
BFNP = ml_dtypes.bfloat16

B, L, D, H, HD = 4, 1024, 1024, 16, 64
NCORES = 8
LQ_C = 512
NP = H // 2  # 8 head pairs
SCALE = 0.125
WS = np.float32(32.0)
INV = float(1.0 / 32.0)

_PROGRAM = None


def build_program():
    nc = bacc.Bacc("TRN2", target_bir_lowering=False, debug=False, num_devices=NCORES)

    def din(name, shape, dt):
        return nc.dram_tensor(name, shape, dt, kind="ExternalInput").ap()

    xq8 = din("xq8", [D, LQ_C], F8)
    dxq8 = din("dxq8", [D, LQ_C], F8)
    xc8 = din("xc8", [D, L], F8)
    dxc8 = din("dxc8", [D, L], F8)
    # Q/K weights packed per pair in SBUF layout: [NP, 128, 8*128]
    wqp = din("wqp", [NP, 128, 1024], F8)
    dwqp = din("dwqp", [NP, 128, 1024], F8)
    wkp = din("wkp", [NP, 128, 1024], F8)
    dwkp = din("dwkp", [NP, 128, 1024], F8)
    wv8 = din("wv8", [D, H * 65], F8)
    dwv8 = din("dwv8", [D, H * 65], F8)
    wob = din("wob", [H * HD, D], BF16)
    bq = din("bq", [128, NP], F32)
    bk = din("bk", [128, NP], F32)
    bo = din("bo", [128, 8], F32)
    bvrow = din("bvrow", [1, H * 65], F32)
    onesd = din("onesd", [1, 128], F32)
    idn = din("idn", [128, 128], BF16)

    out = nc.dram_tensor("out", [D, LQ_C], F32, kind="ExternalOutput").ap()
    resid = nc.dram_tensor("resid", [H * HD, LQ_C], BF16, kind="ExternalOutput").ap()

    def bview(t):
        return t.rearrange("(db p) m -> p db m", p=128)

    xq8_v, dxq8_v = bview(xq8), bview(dxq8)
    xc8_v, dxc8_v = bview(xc8), bview(dxc8)
    wv8_v, dwv8_v = bview(wv8), bview(dwv8)
    wob_v = bview(wob)

    with tile.TileContext(nc) as tc:
        with (
            tc.tile_pool(name="persist", bufs=1) as persist,
            tc.tile_pool(name="wqk", bufs=10) as wqk_pool,
            tc.tile_pool(name="ktp", bufs=3) as kt_pool,
            tc.tile_pool(name="qtp", bufs=3) as qt_pool,
            tc.tile_pool(name="expp", bufs=26) as exp_pool,
            tc.tile_pool(name="rrp", bufs=4) as rr_pool,
            tc.tile_pool(name="osp", bufs=2) as os_pool,
            tc.tile_pool(name="psA", bufs=2, space="PSUM") as psA,
            tc.tile_pool(name="psS", bufs=2, space="PSUM") as psS,
            tc.tile_pool(name="psV", bufs=2, space="PSUM") as psV,
        ):
            def emit_iteration():
                xq_t = persist.tile([128, 8, LQ_C], F8, tag="xq", name="xq_t")
                dxq_t = persist.tile([128, 8, LQ_C], F8, tag="dxq", name="dxq_t")
                xc_t = persist.tile([128, 8, L], F8, tag="xc", name="xc_t")
                dxc_t = persist.tile([128, 8, L], F8, tag="dxc", name="dxc_t")
                wv_t = persist.tile([128, 8, H * 65], F8, tag="wv", name="wv_t")
                dwv_t = persist.tile([128, 8, H * 65], F8, tag="dwv", name="dwv_t")
                wo_t = persist.tile([128, 8, D], BF16, tag="wo", name="wo_t")
                v_all = persist.tile([128, 8, H * 65], BF16, tag="va", name="v_all")
                mflip = persist.tile([128, 4, H * HD], BF16, tag="mf", name="mflip")
                multiT = persist.tile([128, 8, 4, 128], BF16, tag="mT", name="multiT")
                idn_t = persist.tile([128, 128], BF16, tag="idn", name="idn_t")
                bq_t = persist.tile([128, NP], F32, tag="bq", name="bq_t")
                bk_t = persist.tile([128, NP], F32, tag="bk", name="bk_t")
                bo_t = persist.tile([128, 8], F32, tag="bo", name="bo_t")
                ones_t = persist.tile([1, 128], F32R, tag="ones", name="ones_t")
                bvr_t = persist.tile([1, H * 65], F32R, tag="bvr", name="bvr_t")

                qT = [None] * NP
                kT = [None] * NP
                et = [[None] * 8 for _ in range(NP)]

                # ---- critical-path DMAs, in arrival order (SP queue) ----
                nc.sync.dma_start(bq_t[:], bq[:])
                nc.sync.dma_start(bk_t[:], bk[:])
                nc.sync.dma_start(bo_t[:], bo[:])
                nc.sync.dma_start(xq_t[:], xq8_v[:])
                nc.sync.dma_start(dxq_t[:], dxq8_v[:])

                # background loads on gpsimd SWDGE, gated behind the last
                # critical-path load so they don't crowd the DMA engines early
                gate_t = persist.tile([128, 4], F8, tag="gate", name="gate_t")

                def bg_loads():
                    nc.gpsimd.tensor_copy(gate_t[:], dxc_t[:, 7, 1020:1024])
                    nc.gpsimd.dma_start(ones_t[:], onesd[:].bitcast(F32R))
                    nc.gpsimd.dma_start(bvr_t[:], bvrow[:].bitcast(F32R))
                    nc.gpsimd.dma_start(idn_t[:], idn[:])
                    nc.gpsimd.dma_start(wv_t[:], wv8_v[:])
                    nc.gpsimd.dma_start(dwv_t[:], dwv8_v[:])
                    nc.gpsimd.dma_start(wo_t[:, 0:4, :], wob_v[:, 0:4, :])
                    nc.gpsimd.dma_start(wo_t[:, 4:8, :], wob_v[:, 4:8, :])

                def dr3(ps, a_ops, tail=None):
                    n = len(a_ops)
                    for i, (lt, rt) in enumerate(a_ops):
                        nc.tensor.matmul(
                            ps[:], lhsT=lt, rhs=rt,
                            start=(i == 0),
                            stop=(i == n - 1 and tail is None),
                            perf_mode=DR,
                        )
                    if tail is not None:
                        tail(ps)

                def load_wpair(src, dsrc, p, nm):
                    wt = wqk_pool.tile([128, 8, 128], F8, tag="wqk", name=f"{nm}{p}")
                    dwt = wqk_pool.tile([128, 8, 128], F8, tag="wqk", name=f"d{nm}{p}")
                    nc.sync.dma_start(
                        wt[:], src[p].rearrange("p (db c) -> p db c", db=8)
                    )
                    nc.sync.dma_start(
                        dwt[:], dsrc[p].rearrange("p (db c) -> p db c", db=8)
                    )
                    return wt, dwt

                def repack(dst, src, xs):
                    # src [128, n] partition blocks (h0lo, h1lo, h0hi, h1hi)
                    # -> dst [64, 2, n]: slot 0 = lo halves, slot 1 = hi
                    nc.sync.dma_start(dst[:, 0, xs], src[0:64, xs])
                    nc.sync.dma_start(dst[:, 1, xs], src[64:128, xs])

                def emit_q(p):
                    wt, dwt = load_wpair(wqp, dwqp, p, "wq")
                    qps = psA.tile([128, LQ_C], F32, tag="proj", name=f"qps{p}")
                    ops = []
                    for xt, wtt in ((xq_t, wt), (dxq_t, wt), (xq_t, dwt)):
                        for j in range(4):
                            ops.append((wtt[:, 2 * j : 2 * j + 2, :],
                                        xt[:, 2 * j : 2 * j + 2, :]))
                    dr3(qps, ops)
                    rs = qt_pool.tile([128, LQ_C], BF16, tag="rs", name=f"rs{p}")
                    nc.vector.tensor_scalar(
                        rs[:], qps[:], INV, bq_t[:, p : p + 1], MULT, ADD
                    )
                    nc.sync.dma_start(resid[p * 128 : (p + 1) * 128, :], rs[:])
                    q8 = qt_pool.tile([128, LQ_C], F8, tag="qt", name=f"q8{p}")
                    nc.vector.tensor_scalar(
                        q8[:], qps[:], INV, bq_t[:, p : p + 1], MULT, ADD
                    )
                    qT[p] = qt_pool.tile([64, 2, LQ_C], F8, tag="qr", name=f"qr{p}")
                    repack(qT[p], q8, slice(0, LQ_C))

                kw = {}

                def emit_k(p, ch):
                    if ch == 0:
                        kw[p] = (*load_wpair(wkp, dwkp, p, "wk"),
                                 kt_pool.tile([128, L], F8, tag="kt", name=f"k8{p}"))
                        kT[p] = kt_pool.tile([64, 2, L], F8, tag="kr", name=f"kr{p}")
                    wt, dwt, k8 = kw[p]
                    kps = psA.tile([128, 512], F32, tag="proj", name=f"kps{p}{ch}")
                    xs = slice(ch * 512, (ch + 1) * 512)
                    ops = []
                    for xt, wtt in ((xc_t, wt), (dxc_t, wt), (xc_t, dwt)):
                        for j in range(4):
                            ops.append((wtt[:, 2 * j : 2 * j + 2, :],
                                        xt[:, 2 * j : 2 * j + 2, xs]))
                    dr3(kps, ops)
                    nc.vector.tensor_scalar(
                        k8[:, xs], kps[:], INV, bk_t[:, p : p + 1], MULT, ADD
                    )
                    repack(kT[p], k8, xs)

                def emit_vtile(i):
                    ch, kvb = divmod(i, 8)
                    vps = psA.tile([128, 260], F32, tag="proj", name=f"vps{ch}{kvb}")
                    ws = slice(ch * 260, (ch + 1) * 260)
                    ks = slice(kvb * 128, (kvb + 1) * 128)
                    ops = []
                    for xt, wtt in ((xc_t, wv_t), (dxc_t, wv_t), (xc_t, dwv_t)):
                        for j in range(4):
                            ops.append((xt[:, 2 * j : 2 * j + 2, ks],
                                        wtt[:, 2 * j : 2 * j + 2, ws]))

                    def vbias(ps):
                        nc.tensor.matmul(
                            ps[:], lhsT=ones_t[0:1, :], rhs=bvr_t[0:1, ws],
                            start=False, stop=True,
                        )

                    dr3(vps, ops, tail=vbias)
                    nc.vector.tensor_scalar_mul(v_all[:, kvb, ws], vps[:], INV)

                def emit_scores(p, kvb):
                    sps = psS.tile([128, 1024], F32, tag="sps", name=f"sps{p}{kvb}")
                    ks = slice(kvb * 128, (kvb + 1) * 128)
                    for h in range(2):
                        hs = slice(32 * h, 32 * h + 32)
                        nc.tensor.matmul(
                            sps[:, h * 512 : (h + 1) * 512],
                            lhsT=kT[p][hs, :, ks],
                            rhs=qT[p][hs, :, :],
                            start=True, stop=True,
                            perf_mode=DR,
                        )
                    e = exp_pool.tile([128, 1024], BF16, tag="expt", name=f"et{p}{kvb}")
                    nc.scalar.activation(e[:], sps[:], EXP, scale=SCALE)
                    et[p][kvb] = e

                def emit_pv_unit(p, qs):
                    pvp = psV.tile([128, 2, 128], F32, tag="pv", name=f"pv{p}{qs}")
                    for h in range(2):
                        for kvb in range(8):
                            nc.tensor.matmul(
                                pvp[:, h, 0:65],
                                lhsT=et[p][kvb][
                                    :, h * 512 + qs * 128 : h * 512 + (qs + 1) * 128
                                ],
                                rhs=v_all[:, kvb,
                                          (2 * p + h) * 65 : (2 * p + h + 1) * 65],
                                start=(kvb == 0),
                                stop=(kvb == 7),
                            )
                    rr = rr_pool.tile([128, 2, 1], F32, tag="rr", name=f"rr{p}{qs}")
                    nc.vector.reciprocal(rr[:], pvp[:, :, 64:65])
                    for h in range(2):
                        nc.vector.tensor_scalar_mul(
                            mflip[:, qs, p * 128 + h * 64 : p * 128 + (h + 1) * 64],
                            pvp[:, h, 0:64],
                            rr[:, h, 0:1],
                        )

                def emit_transpose(p):
                    tp = psA.tile([128, 4, 128], BF16, tag="proj", name=f"tp{p}")
                    for qs in range(4):
                        nc.tensor.matmul(
                            tp[:, qs, :],
                            lhsT=mflip[:, qs, p * 128 : (p + 1) * 128],
                            rhs=idn_t[:],
                            is_transpose=True,
                        )
                    nc.vector.tensor_copy(multiT[:, p, :, :], tp[:])

                def emit_oproj():
                    for db in range(8):
                        pool, tg = (psS, "sps") if db % 2 == 0 else (psA, "proj")
                        ops = pool.tile([128, 512], F32, tag=tg, name=f"ops{db}")
                        for hcb in range(8):
                            nc.tensor.matmul(
                                ops[:],
                                lhsT=wo_t[:, hcb, db * 128 : (db + 1) * 128],
                                rhs=multiT[:, hcb, :, :],
                                start=(hcb == 0),
                                stop=(hcb == 7),
                            )
                        osb = os_pool.tile([128, 512], F32, tag="outsb", name=f"ob{db}")
                        nc.vector.tensor_scalar(
                            osb[:], ops[:], INV, bo_t[:, db : db + 1], MULT, ADD
                        )
                        nc.sync.dma_start(out[db * 128 : (db + 1) * 128, :], osb[:])

                # ---------------- schedule ----------------
                emit_q(0)
                nc.sync.dma_start(xc_t[:, :, 0:512], xc8_v[:, :, 0:512])
                nc.sync.dma_start(dxc_t[:, :, 0:512], dxc8_v[:, :, 0:512])
                emit_k(0, 0)
                nc.sync.dma_start(xc_t[:, :, 512:1024], xc8_v[:, :, 512:1024])
                nc.sync.dma_start(dxc_t[:, :, 512:1024], dxc8_v[:, :, 512:1024])
                bg_loads()

                vt_next = 0

                def emit_vtiles(nmax):
                    nonlocal vt_next
                    while vt_next < min(nmax, 32):
                        emit_vtile(vt_next)
                        vt_next += 1

                def vtarget(p, kvb):
                    if p == 0:
                        return 0
                    if p == 1:
                        return min(8, max(0, 3 * (kvb - 1)))
                    s2 = 8 * (p - 2) + kvb
                    base = 8 + ((s2 + 1) * 24 + 27) // 28
                    base = max(base, 8 * ((p - 2) // 2 + 1))
                    if kvb >= 3:
                        base = max(base, 8 * ((p - 1) // 2 + 1))
                    return min(32, base)

                for p in range(NP):
                    for kvb in range(8):
                        emit_scores(p, kvb)
                        if p == 0 and kvb == 3:
                            emit_k(0, 1)
                        if p + 1 < NP:
                            if kvb == 1:
                                emit_q(p + 1)
                            if kvb == 3:
                                emit_k(p + 1, 0)
                                emit_k(p + 1, 1)
                        emit_vtiles(vtarget(p, kvb))
                        if 2 <= p and kvb < 4:
                            emit_pv_unit(p - 2, kvb)
                        if p >= 2 and kvb == 4:
                            emit_transpose(p - 2)
                        if p == 7 and kvb >= 4:
                            emit_pv_unit(6, kvb - 4)
                    if p == 7:
                        emit_transpose(6)
                for qs in range(4):
                    emit_pv_unit(7, qs)
                emit_transpose(7)
                emit_oproj()

            with nc.allow_low_precision(reason="delta-fp8 kernel"):
                emit_iteration()

    nc.compile()
    return nc


def _q8pair(x):
    x = np.ascontiguousarray(np.asarray(x, np.float32))
    a = x.astype(E4NP)
    d = (x - a.astype(np.float32)).astype(E4NP)
    return a, d


def _packpairs(w):
    # [D, 1024] -> [NP, 128, 1024] so pair p's slice is one clean DMA:
    # packed[p, part, db*128+c] = w[db*128+part, p*128+c]
    return np.ascontiguousarray(
        w.reshape(8, 128, NP, 128).transpose(2, 1, 0, 3).reshape(NP, 128, 1024)
    )


# within each pair's 128 cols: shuffled s = 64*half + 32*head + r maps to
# true col 64*head + 32*half + r  (lo halves of both heads first)
_SHUF = np.array([64 * h + 32 * a + r
                  for a in (0, 1) for h in (0, 1) for r in range(32)])


def _shuf_cols(w):
    # w [D, H*HD] head-major; permute each pair's 128-col block
    v = w.reshape(D, NP, 128)
    return np.ascontiguousarray(v[:, :, _SHUF].reshape(D, H * HD))


def _shuf_rows(b):
    # b [128, NP] per-pair bias cols in true order -> shuffled partition order
    v = b.T  # [NP, 128]
    return np.ascontiguousarray(v[:, _SHUF].T)


def _marshal(inputs):
    q = np.ascontiguousarray(np.asarray(inputs["queries"], dtype=np.float32))
    c = np.ascontiguousarray(np.asarray(inputs["context"], dtype=np.float32))
    Wq = np.asarray(inputs["Wq"], dtype=np.float32)
    Wk = np.asarray(inputs["Wk"], dtype=np.float32)
    Wv = np.asarray(inputs["Wv"], dtype=np.float32)
    Wo = np.ascontiguousarray(np.asarray(inputs["Wo"], dtype=np.float32))
    bq = np.asarray(inputs["bq"], dtype=np.float32)
    bk = np.asarray(inputs["bk"], dtype=np.float32)
    bv = np.asarray(inputs["bv"], dtype=np.float32)
    bo = np.asarray(inputs["bo"], dtype=np.float32)

    wq8, dwq8 = _q8pair(_shuf_cols(WS * Wq.transpose(1, 0, 2).reshape(D, H * HD)))
    wk8, dwk8 = _q8pair(_shuf_cols(WS * Wk.transpose(1, 0, 2).reshape(D, H * HD)))
    wv_aug = np.zeros((D, H, 65), np.float32)
    wv_aug[:, :, :64] = Wv.transpose(1, 0, 2).reshape(D, H, HD)
    wv8, dwv8 = _q8pair(WS * wv_aug.reshape(D, H * 65))
    # bias row: [32*bv_h | 1.0] per head; v psum = 32*(v+bv), den col = 1.0
    bv_aug = np.full((H, 65), 1.0, np.float32)
    bv_aug[:, :64] = 32.0 * bv.reshape(H, HD)

    shared = {
        "wqp": _packpairs(wq8.astype(np.float32)).astype(E4NP),
        "dwqp": _packpairs(dwq8.astype(np.float32)).astype(E4NP),
        "wkp": _packpairs(wk8.astype(np.float32)).astype(E4NP),
        "dwkp": _packpairs(dwk8.astype(np.float32)).astype(E4NP),
        "wv8": wv8, "dwv8": dwv8,
        "wob": Wo.astype(BFNP),
        "bq": _shuf_rows(np.ascontiguousarray(bq.reshape(NP, 128).T)),
        "bk": _shuf_rows(np.ascontiguousarray(bk.reshape(NP, 128).T)),
        "bo": np.ascontiguousarray(bo.reshape(8, 128).T),
        "bvrow": np.ascontiguousarray(bv_aug.reshape(1, H * 65)),
        "onesd": np.ones((1, 128), np.float32),
        "idn": np.eye(128, dtype=BFNP),
    }
    in_maps = []
    for core in range(NCORES):
        b, half = core // 2, core % 2
        m = dict(shared)
        xq8c, dxq8c = _q8pair(q[b].T[:, half * LQ_C : (half + 1) * LQ_C])
        xc8c, dxc8c = _q8pair(c[b].T)
        m["xq8"], m["dxq8"] = xq8c, dxq8c
        m["xc8"], m["dxc8"] = xc8c, dxc8c
        in_maps.append(m)
    return in_maps


def kernel(**inputs):
    global _PROGRAM
    if _PROGRAM is None:
        _PROGRAM = build_program()
    in_maps = _marshal(inputs)
    res = run_bass_kernel_spmd(_PROGRAM, in_maps, list(range(NCORES)))
    out = np.empty((B, L, D), np.float32)
    residual = np.empty((B, L, H * HD), np.float32)
    unshuf = np.concatenate([p * 128 + _SHUF for p in range(NP)])
    for core in range(NCORES):
        b, half = core // 2, core % 2
        sl = slice(half * LQ_C, (half + 1) * LQ_C)
        out[b, sl, :] = res.results[core]["out"].T
        raw = np.asarray(res.results[core]["resid"], np.float32)
        rt = np.empty_like(raw)
        rt[unshuf, :] = raw
        residual[b, sl, :] = rt.T
    return out, residual


# revision 3
# speedup vs baseline: 1.0347x; 1.0030x over previous
"""Trainium2 Bass kernel for nn_MedPoseAttention — v1 (delta-fp8 projections).

Sharding: 8 cores = 4 batches x 2 query halves; each core does full context
K/V (replicated within the pair), all 16 heads, 512 query rows.

Per-core dataflow:
  Q/K/V projections: two-level fp8e4m3 DoubleRow (x8*w8 + dx8*w8 + x8*dw8),
    host-prepared operands, weights pre-scaled x32. 3 passes x 4 DR matmuls
    (K=256 each) = 0.75x the bf16 matmul cost at ~bf16 accuracy.
  resid = q_psum/32 + bq  (DVE tensor_scalar, bf16 out, DMA'd directly)
  scores = qT_bf . kT_bf per head (bf16, PE quadrant packing, K=64)
  exp: ACT activation Exp -> bf16 tiles [128kv, 2heads*512q]
  PV flipped: out[q, 65] per (pair-head, q-slice); col 64 = denominator
    (v_aug 65th col = 1.0 in the f32r bias ones-row; v_bf = psum/32).
  norm: R = 1/den (per-partition!); multi = pv * R (tensor_scalar, bf16)
    = 32*(head_out + bv) -> mflip [q, hc]
  transpose: PE identity-transpose mflip -> multiT [hc, q] (bf16)
  O-proj: out[Dslice, q] = multiT.T @ Wo_bf16; out = psum/32 + bo (per-part).
Weight slices for Q/K are host-packed per head-pair in SBUF layout so each
pair's 128KB slice arrives as one clean DMA just before it is needed.
"""

import sys

if "/opt/trn_rl_repo" not in sys.path:
    sys.path.insert(0, "/opt/trn_rl_repo")

import numpy as np
import ml_dtypes

import concourse.bass as bass  # noqa: F401
import concourse.mybir as mybir
from concourse import bacc, tile
from concourse.bass_utils import run_bass_kernel_spmd

F32 = mybir.dt.float32
F32R = mybir.dt.float32r
BF16 = mybir.dt.bfloat16
F8 = mybir.dt.float8e4
DR = mybir.MatmulPerfMode.DoubleRow
MULT = mybir.AluOpType.mult
ADD = mybir.AluOpType.add
EXP = mybir.ActivationFunctionType.Exp

E4NP = ml_dtypes.float8_e4m3
BFNP = ml_dtypes.bfloat16

B, L, D, H, HD = 4, 1024, 1024, 16, 64
NCORES = 8
LQ_C = 512
NP = H // 2  # 8 head pairs
SCALE = 0.125
WS = np.float32(32.0)
INV = float(1.0 / 32.0)

_PROGRAM = None


def build_program():
    nc = bacc.Bacc("TRN2", target_bir_lowering=False, debug=False, num_devices=NCORES)

    def din(name, shape, dt):
        return nc.dram_tensor(name, shape, dt, kind="ExternalInput").ap()

    xq8 = din("xq8", [D, LQ_C], F8)
    dxq8 = din("dxq8", [D, LQ_C], F8)
    xc8 = din("xc8", [D, L], F8)
    dxc8 = din("dxc8", [D, L], F8)
    # Q/K weights packed per pair in SBUF layout: [NP, 128, 8*128]
    wqp = din("wqp", [NP, 128, 1024], F8)
    dwqp = din("dwqp", [NP, 128, 1024], F8)
    wkp = din("wkp", [NP, 128, 1024], F8)
    dwkp = din("dwkp", [NP, 128, 1024], F8)
    wv8 = din("wv8", [D, H * 65], F8)
    dwv8 = din("dwv8", [D, H * 65], F8)
    wob = din("wob", [H * HD, D], BF16)
    bq = din("bq", [128, NP], F32)
    bk = din("bk", [128, NP], F32)
    bo = din("bo", [128, 8], F32)
    bvrow = din("bvrow", [1, H * 65], F32)
    onesd = din("onesd", [1, 128], F32)
    idn = din("idn", [128, 128], BF16)

    out = nc.dram_tensor("out", [D, LQ_C], F32, kind="ExternalOutput").ap()
    resid = nc.dram_tensor("resid", [H * HD, LQ_C], BF16, kind="ExternalOutput").ap()

    def bview(t):
        return t.rearrange("(db p) m -> p db m", p=128)

    xq8_v, dxq8_v = bview(xq8), bview(dxq8)
    xc8_v, dxc8_v = bview(xc8), bview(dxc8)
    wv8_v, dwv8_v = bview(wv8), bview(dwv8)
    wob_v = bview(wob)

    with tile.TileContext(nc) as tc:
        with (
            tc.tile_pool(name="persist", bufs=1) as persist,
            tc.tile_pool(name="wqk", bufs=10) as wqk_pool,
            tc.tile_pool(name="ktp", bufs=3) as kt_pool,
            tc.tile_pool(name="qtp", bufs=3) as qt_pool,
            tc.tile_pool(name="expp", bufs=26) as exp_pool,
            tc.tile_pool(name="rrp", bufs=4) as rr_pool,
            tc.tile_pool(name="osp", bufs=2) as os_pool,
            tc.tile_pool(name="psA", bufs=2, space="PSUM") as psA,
            tc.tile_pool(name="psS", bufs=2, space="PSUM") as psS,
            tc.tile_pool(name="psV", bufs=2, space="PSUM") as psV,
        ):
            def emit_iteration():
                xq_t = persist.tile([128, 8, LQ_C], F8, tag="xq", name="xq_t")
                dxq_t = persist.tile([128, 8, LQ_C], F8, tag="dxq", name="dxq_t")
                xc_t = persist.tile([128, 8, L], F8, tag="xc", name="xc_t")
                dxc_t = persist.tile([128, 8, L], F8, tag="dxc", name="dxc_t")
                wv_t = persist.tile([128, 8, H * 65], F8, tag="wv", name="wv_t")
                dwv_t = persist.tile([128, 8, H * 65], F8, tag="dwv", name="dwv_t")
                wo_t = persist.tile([128, 8, D], BF16, tag="wo", name="wo_t")
                v_all = persist.tile([128, 8, H * 65], BF16, tag="va", name="v_all")
                mflip = persist.tile([128, 4, H * HD], BF16, tag="mf", name="mflip")
                multiT = persist.tile([128, 8, 4, 128], BF16, tag="mT", name="multiT")
                idn_t = persist.tile([128, 128], BF16, tag="idn", name="idn_t")
                bq_t = persist.tile([128, NP], F32, tag="bq", name="bq_t")
                bk_t = persist.tile([128, NP], F32, tag="bk", name="bk_t")
                bo_t = persist.tile([128, 8], F32, tag="bo", name="bo_t")
                ones_t = persist.tile([1, 128], F32R, tag="ones", name="ones_t")
                bvr_t = persist.tile([1, H * 65], F32R, tag="bvr", name="bvr_t")

                qT = [None] * NP
                kT = [None] * NP
                et = [[None] * 8 for _ in range(NP)]

                # ---- critical-path DMAs, in arrival order (SP queue) ----
                nc.sync.dma_start(bq_t[:], bq[:])
                nc.sync.dma_start(bk_t[:], bk[:])
                nc.sync.dma_start(bo_t[:], bo[:])
                nc.sync.dma_start(xq_t[:], xq8_v[:])
                nc.sync.dma_start(dxq_t[:], dxq8_v[:])

                # background loads on gpsimd SWDGE, gated behind the last
                # critical-path load so they don't crowd the DMA engines early
                gate_t = persist.tile([128, 4], F8, tag="gate", name="gate_t")

                def bg_loads():
                    nc.gpsimd.tensor_copy(gate_t[:], dxc_t[:, 7, 1020:1024])
                    nc.gpsimd.dma_start(ones_t[:], onesd[:].bitcast(F32R))
                    nc.gpsimd.dma_start(bvr_t[:], bvrow[:].bitcast(F32R))
                    nc.gpsimd.dma_start(idn_t[:], idn[:])
                    nc.gpsimd.dma_start(wv_t[:], wv8_v[:])
                    nc.gpsimd.dma_start(dwv_t[:], dwv8_v[:])
                    nc.gpsimd.dma_start(wo_t[:, 0:4, :], wob_v[:, 0:4, :])
                    nc.gpsimd.dma_start(wo_t[:, 4:8, :], wob_v[:, 4:8, :])

                def dr3(ps, a_ops, tail=None):
                    n = len(a_ops)
                    for i, (lt, rt) in enumerate(a_ops):
                        nc.tensor.matmul(
                            ps[:], lhsT=lt, rhs=rt,
                            start=(i == 0),
                            stop=(i == n - 1 and tail is None),
                            perf_mode=DR,
                        )
                    if tail is not None:
                        tail(ps)

                def load_wpair(src, dsrc, p, nm):
                    wt = wqk_pool.tile([128, 8, 128], F8, tag="wqk", name=f"{nm}{p}")
                    dwt = wqk_pool.tile([128, 8, 128], F8, tag="wqk", name=f"d{nm}{p}")
                    nc.gpsimd.dma_start(
                        wt[:], src[p].rearrange("p (db c) -> p db c", db=8)
                    )
                    nc.gpsimd.dma_start(
                        dwt[:], dsrc[p].rearrange("p (db c) -> p db c", db=8)
                    )
                    return wt, dwt

                def repack(dst, src, xs):
                    # src [128, n] partition blocks (h0lo, h1lo, h0hi, h1hi)
                    # -> dst [64, 2, n]: slot 0 = lo halves, slot 1 = hi
                    nc.sync.dma_start(dst[:, 0, xs], src[0:64, xs])
                    nc.sync.dma_start(dst[:, 1, xs], src[64:128, xs])

                def emit_q(p, lw=None):
                    wt, dwt = lw if lw is not None else load_wpair(wqp, dwqp, p, "wq")
                    qps = psA.tile([128, LQ_C], F32, tag="proj", name=f"qps{p}")
                    ops = []
                    for xt, wtt in ((xq_t, wt), (dxq_t, wt), (xq_t, dwt)):
                        for j in range(4):
                            ops.append((wtt[:, 2 * j : 2 * j + 2, :],
                                        xt[:, 2 * j : 2 * j + 2, :]))
                    dr3(qps, ops)
                    rs = qt_pool.tile([128, LQ_C], BF16, tag="rs", name=f"rs{p}")
                    nc.vector.tensor_scalar(
                        rs[:], qps[:], INV, bq_t[:, p : p + 1], MULT, ADD
                    )
                    nc.sync.dma_start(resid[p * 128 : (p + 1) * 128, :], rs[:])
                    q8 = qt_pool.tile([128, LQ_C], F8, tag="qt", name=f"q8{p}")
                    nc.vector.tensor_scalar(
                        q8[:], qps[:], INV, bq_t[:, p : p + 1], MULT, ADD
                    )
                    qT[p] = qt_pool.tile([64, 2, LQ_C], F8, tag="qr", name=f"qr{p}")
                    repack(qT[p], q8, slice(0, LQ_C))

                kw = {}

                def emit_k(p, ch, lw=None):
                    if ch == 0:
                        kw[p] = (*(lw if lw is not None else load_wpair(wkp, dwkp, p, "wk")),
                                 kt_pool.tile([128, L], F8, tag="kt", name=f"k8{p}"))
                        kT[p] = kt_pool.tile([64, 2, L], F8, tag="kr", name=f"kr{p}")
                    wt, dwt, k8 = kw[p]
                    kps = psA.tile([128, 512], F32, tag="proj", name=f"kps{p}{ch}")
                    xs = slice(ch * 512, (ch + 1) * 512)
                    ops = []
                    for xt, wtt in ((xc_t, wt), (dxc_t, wt), (xc_t, dwt)):
                        for j in range(4):
                            ops.append((wtt[:, 2 * j : 2 * j + 2, :],
                                        xt[:, 2 * j : 2 * j + 2, xs]))
                    dr3(kps, ops)
                    nc.vector.tensor_scalar(
                        k8[:, xs], kps[:], INV, bk_t[:, p : p + 1], MULT, ADD
                    )
                    repack(kT[p], k8, xs)

                def emit_vtile(i):
                    ch, kvb = divmod(i, 8)
                    vps = psA.tile([128, 260], F32, tag="proj", name=f"vps{ch}{kvb}")
                    ws = slice(ch * 260, (ch + 1) * 260)
                    ks = slice(kvb * 128, (kvb + 1) * 128)
                    ops = []
                    for xt, wtt in ((xc_t, wv_t), (dxc_t, wv_t), (xc_t, dwv_t)):
                        for j in range(4):
                            ops.append((xt[:, 2 * j : 2 * j + 2, ks],
                                        wtt[:, 2 * j : 2 * j + 2, ws]))

                    def vbias(ps):
                        nc.tensor.matmul(
                            ps[:], lhsT=ones_t[0:1, :], rhs=bvr_t[0:1, ws],
                            start=False, stop=True,
                        )

                    dr3(vps, ops, tail=vbias)
                    nc.vector.tensor_scalar_mul(v_all[:, kvb, ws], vps[:], INV)

                def emit_scores(p, kvb):
                    sps = psS.tile([128, 1024], F32, tag="sps", name=f"sps{p}{kvb}")
                    ks = slice(kvb * 128, (kvb + 1) * 128)
                    for h in range(2):
                        hs = slice(32 * h, 32 * h + 32)
                        nc.tensor.matmul(
                            sps[:, h * 512 : (h + 1) * 512],
                            lhsT=kT[p][hs, :, ks],
                            rhs=qT[p][hs, :, :],
                            start=True, stop=True,
                            perf_mode=DR,
                        )
                    e = exp_pool.tile([128, 1024], BF16, tag="expt", name=f"et{p}{kvb}")
                    nc.scalar.activation(e[:], sps[:], EXP, scale=SCALE)
                    et[p][kvb] = e

                def emit_pv_unit(p, qs):
                    pvp = psV.tile([128, 2, 128], F32, tag="pv", name=f"pv{p}{qs}")
                    for h in range(2):
                        for kvb in range(8):
                            nc.tensor.matmul(
                                pvp[:, h, 0:65],
                                lhsT=et[p][kvb][
                                    :, h * 512 + qs * 128 : h * 512 + (qs + 1) * 128
                                ],
                                rhs=v_all[:, kvb,
                                          (2 * p + h) * 65 : (2 * p + h + 1) * 65],
                                start=(kvb == 0),
                                stop=(kvb == 7),
                            )
                    rr = rr_pool.tile([128, 2, 1], F32, tag="rr", name=f"rr{p}{qs}")
                    nc.vector.reciprocal(rr[:], pvp[:, :, 64:65])
                    for h in range(2):
                        nc.vector.tensor_scalar_mul(
                            mflip[:, qs, p * 128 + h * 64 : p * 128 + (h + 1) * 64],
                            pvp[:, h, 0:64],
                            rr[:, h, 0:1],
                        )

                def emit_transpose(p):
                    tp = psA.tile([128, 4, 128], BF16, tag="proj", name=f"tp{p}")
                    for qs in range(4):
                        nc.tensor.matmul(
                            tp[:, qs, :],
                            lhsT=mflip[:, qs, p * 128 : (p + 1) * 128],
                            rhs=idn_t[:],
                            is_transpose=True,
                        )
                    nc.vector.tensor_copy(multiT[:, p, :, :], tp[:])

                def emit_oproj():
                    for db in range(8):
                        pool, tg = (psS, "sps") if db % 2 == 0 else (psA, "proj")
                        ops = pool.tile([128, 512], F32, tag=tg, name=f"ops{db}")
                        for hcb in range(8):
                            nc.tensor.matmul(
                                ops[:],
                                lhsT=wo_t[:, hcb, db * 128 : (db + 1) * 128],
                                rhs=multiT[:, hcb, :, :],
                                start=(hcb == 0),
                                stop=(hcb == 7),
                            )
                        osb = os_pool.tile([128, 512], F32, tag="outsb", name=f"ob{db}")
                        nc.vector.tensor_scalar(
                            osb[:], ops[:], INV, bo_t[:, db : db + 1], MULT, ADD
                        )
                        nc.sync.dma_start(out[db * 128 : (db + 1) * 128, :], osb[:])

                # ---------------- schedule ----------------
                lw_q0 = load_wpair(wqp, dwqp, 0, "wq")
                nc.sync.dma_start(xc_t[:, :, 0:512], xc8_v[:, :, 0:512])
                nc.sync.dma_start(dxc_t[:, :, 0:512], dxc8_v[:, :, 0:512])
                lw_k0 = load_wpair(wkp, dwkp, 0, "wk")
                nc.sync.dma_start(xc_t[:, :, 512:1024], xc8_v[:, :, 512:1024])
                nc.sync.dma_start(dxc_t[:, :, 512:1024], dxc8_v[:, :, 512:1024])
                bg_loads()
                emit_q(0, lw_q0)
                emit_k(0, 0, lw_k0)

                vt_next = 0

                def emit_vtiles(nmax):
                    nonlocal vt_next
                    while vt_next < min(nmax, 32):
                        emit_vtile(vt_next)
                        vt_next += 1

                def vtarget(p, kvb):
                    if p == 0:
                        return 0
                    if p == 1:
                        return min(8, max(0, 3 * (kvb - 1)))
                    s2 = 8 * (p - 2) + kvb
                    base = 8 + ((s2 + 1) * 24 + 27) // 28
                    base = max(base, 8 * ((p - 2) // 2 + 1))
                    if kvb >= 3:
                        base = max(base, 8 * ((p - 1) // 2 + 1))
                    return min(32, base)

                for p in range(NP):
                    for kvb in range(8):
                        emit_scores(p, kvb)
                        if p == 0 and kvb == 3:
                            emit_k(0, 1)
                        if p + 1 < NP:
                            if kvb == 1:
                                emit_q(p + 1)
                            if kvb == 3:
                                emit_k(p + 1, 0)
                                emit_k(p + 1, 1)
                        emit_vtiles(vtarget(p, kvb))
                        if 2 <= p and kvb < 4:
                            emit_pv_unit(p - 2, kvb)
                        if p >= 2 and kvb == 4:
                            emit_transpose(p - 2)
                        if p == 7 and kvb >= 4:
                            emit_pv_unit(6, kvb - 4)
                    if p == 7:
                        emit_transpose(6)
                for qs in range(4):
                    emit_pv_unit(7, qs)
                emit_transpose(7)
                emit_oproj()

            with nc.allow_low_precision(reason="delta-fp8 kernel"):
                emit_iteration()

    nc.compile()
    return nc


def _q8pair(x):
    x = np.ascontiguousarray(np.asarray(x, np.float32))
    a = x.astype(E4NP)
    d = (x - a.astype(np.float32)).astype(E4NP)
    return a, d


def _packpairs(w):
    # [D, 1024] -> [NP, 128, 1024] so pair p's slice is one clean DMA:
    # packed[p, part, db*128+c] = w[db*128+part, p*128+c]
    return np.ascontiguousarray(
        w.reshape(8, 128, NP, 128).transpose(2, 1, 0, 3).reshape(NP, 128, 1024)
    )


# within each pair's 128 cols: shuffled s = 64*half + 32*head + r maps to
# true col 64*head + 32*half + r  (lo halves of both heads first)
_SHUF = np.array([64 * h + 32 * a + r
                  for a in (0, 1) for h in (0, 1) for r in range(32)])


def _shuf_cols(w):
    # w [D, H*HD] head-major; permute each pair's 128-col block
    v = w.reshape(D, NP, 128)
    return np.ascontiguousarray(v[:, :, _SHUF].reshape(D, H * HD))


def _shuf_rows(b):
    # b [128, NP] per-pair bias cols in true order -> shuffled partition order
    v = b.T  # [NP, 128]
    return np.ascontiguousarray(v[:, _SHUF].T)


def _marshal(inputs):
    q = np.ascontiguousarray(np.asarray(inputs["queries"], dtype=np.float32))
    c = np.ascontiguousarray(np.asarray(inputs["context"], dtype=np.float32))
    Wq = np.asarray(inputs["Wq"], dtype=np.float32)
    Wk = np.asarray(inputs["Wk"], dtype=np.float32)
    Wv = np.asarray(inputs["Wv"], dtype=np.float32)
    Wo = np.ascontiguousarray(np.asarray(inputs["Wo"], dtype=np.float32))
    bq = np.asarray(inputs["bq"], dtype=np.float32)
    bk = np.asarray(inputs["bk"], dtype=np.float32)
    bv = np.asarray(inputs["bv"], dtype=np.float32)
    bo = np.asarray(inputs["bo"], dtype=np.float32)

    wq8, dwq8 = _q8pair(_shuf_cols(WS * Wq.transpose(1, 0, 2).reshape(D, H * HD)))
    wk8, dwk8 = _q8pair(_shuf_cols(WS * Wk.transpose(1, 0, 2).reshape(D, H * HD)))
    wv_aug = np.zeros((D, H, 65), np.float32)
    wv_aug[:, :, :64] = Wv.transpose(1, 0, 2).reshape(D, H, HD)
    wv8, dwv8 = _q8pair(WS * wv_aug.reshape(D, H * 65))
    # bias row: [32*bv_h | 1.0] per head; v psum = 32*(v+bv), den col = 1.0
    bv_aug = np.full((H, 65), 1.0, np.float32)
    bv_aug[:, :64] = 32.0 * bv.reshape(H, HD)

    shared = {
        "wqp": _packpairs(wq8.astype(np.float32)).astype(E4NP),
        "dwqp": _packpairs(dwq8.astype(np.float32)).astype(E4NP),
        "wkp": _packpairs(wk8.astype(np.float32)).astype(E4NP),
        "dwkp": _packpairs(dwk8.astype(np.float32)).astype(E4NP),
        "wv8": wv8, "dwv8": dwv8,
        "wob": Wo.astype(BFNP),
        "bq": _shuf_rows(np.ascontiguousarray(bq.reshape(NP, 128).T)),
        "bk": _shuf_rows(np.ascontiguousarray(bk.reshape(NP, 128).T)),
        "bo": np.ascontiguousarray(bo.reshape(8, 128).T),
        "bvrow": np.ascontiguousarray(bv_aug.reshape(1, H * 65)),
        "onesd": np.ones((1, 128), np.float32),
        "idn": np.eye(128, dtype=BFNP),
    }
    in_maps = []
    for core in range(NCORES):
        b, half = core // 2, core % 2
        m = dict(shared)
        xq8c, dxq8c = _q8pair(q[b].T[:, half * LQ_C : (half + 1) * LQ_C])
        xc8c, dxc8c = _q8pair(c[b].T)
        m["xq8"], m["dxq8"] = xq8c, dxq8c
        m["xc8"], m["dxc8"] = xc8c, dxc8c
        in_maps.append(m)
    return in_maps


def kernel(**inputs):
    global _PROGRAM
    if _PROGRAM is None:
        _PROGRAM = build_program()
    in_maps = _marshal(inputs)
    res = run_bass_kernel_spmd(_PROGRAM, in_maps, list(range(NCORES)))
    out = np.empty((B, L, D), np.float32)
    residual = np.empty((B, L, H * HD), np.float32)
    unshuf = np.concatenate([p * 128 + _SHUF for p in range(NP)])
    for core in range(NCORES):
        b, half = core // 2, core % 2
        sl = slice(half * LQ_C, (half + 1) * LQ_C)
        out[b, sl, :] = res.results[core]["out"].T
        raw = np.asarray(res.results[core]["resid"], np.float32)
        rt = np.empty_like(raw)
        rt[unshuf, :] = raw
        residual[b, sl, :] = rt.T
    return out, residual
